# revision 6
# baseline (speedup 1.0000x reference)
"""Trainium2 Bass kernel for nn_Model_39676907882504.

Math: qk = (q @ k^T)/8 has shape [1,2048,1,1]; after the transposes it is
[2048,1,1,1], and softmax over the trailing size-1 axis is exactly 1.0
regardless of qk (exp(x-max)/sum == 1/1 bit-exactly). The final matmul
[S,Q,B,Q] @ [B,S,Q,D] with attn_weight == 1 therefore reduces to
broadcasting `value` across a new leading dim:

    output[i, j, 0, :] = value[0, j, 0, :]   for all i in [0, 2048)

i.e. a 512KB -> 1GiB broadcast copy.  Pure memory-regime kernel.

Sharding (per the hint): leading output dim (2048 rows) split across the
8 cores, 256 rows/core; value replicated.

HW model (from trace analysis + an engine-assignment probe):
  - HWDGE splits each DMA instruction's descriptors positionally over the
    16 SDMA engines: descriptor i -> engine 64+(i%16), restarting at 64
    for every instruction.  SBUF AXI port p serves partitions ≡ p (mod
    16), so descriptor i must read partition ≡ i (mod 16) or two engines
    share a port and halve their rate.
  - Engines 64-78 each sustain ~26.9 GB/s (99% of the 27.2 GB/s port
    rate).  Engine 79 only sustains ~21.4 GB/s (known "engine 15 slower"
    silicon quirk) and the old 128-desc stores made everything wait ~70us
    for it.
  - HBM reads of the same 512KB region from many engines at once run
    latency-bound (~7 GB/s/engine), so the old 8x replicated load burned
    ~35us.

Kernel: value is staged in SBUF as 15 overlapping windows of C=8740
floats (stride 8738; window p = vflat[8738p : 8738p+8740]), one window
per partition.  Each output row (131072 floats) is written by ONE
15-descriptor store instruction: descriptor p writes window p to row
offset 8738p.  Adjacent descriptors overlap by 2 floats (same data,
harmless).  15 descs -> engines 64-78 only, partition p ≡ engine p:
port-aligned, slow engine 79 idle.  256 rows/core split across both
HWDGE queues (sync + scalar).
"""

import sys

for _p in ("/opt/trn_rl_repo",):
    if _p not in sys.path:
        sys.path.insert(0, _p)

import numpy as np

import bass_rust
import concourse.bass as bass
import concourse.mybir as mybir
from concourse.bass_utils import run_bass_kernel_spmd

S = 2048
D = 64
N_CORES = 8
ROWS_PER_CORE = S // N_CORES          # 256
ROW_FL = S * D                        # 131072 floats per output row
NP = 15                               # store descriptors per instruction
C = 8740                              # floats per descriptor (34960 B)
STRIDE = 8738                         # row stride between descriptors
assert (NP - 1) * STRIDE + C == ROW_FL

TRACE = False          # test.py flips this to profile
TRACE_KWARGS = {}
LAST_RESULT = None     # BassKernelResults of the last run (for test.py)


def build_program():
    nc = bass.Bass()
    val = nc.declare_dram_parameter("value15", [16, C], mybir.dt.float32,
                                    isOutput=False)
    out = nc.declare_dram_parameter("out", [ROWS_PER_CORE, ROW_FL],
                                    mybir.dt.float32, isOutput=True)
    vtile = nc.alloc_sbuf_tensor("vtile", [16, C], mybir.dt.float32)

    half = ROWS_PER_CORE // 2

    def store_row(eng, r):
        o = out[r:r + 1, 0:ROW_FL]
        # overlapping windows; max extent = ROW_FL exactly
        o.ap = bass_rust.VecI64Pair([[STRIDE, NP], [1, C]])
        return eng.dma_start(out=o, in_=vtile[0:NP, 0:C])

    def closer_row(eng, r):
        # 16 x 1-float rewrite of row r positions {8738p}: descriptor p
        # is FIFO-behind every earlier descriptor on engine p for this
        # queue, so then_inc on this vouches for the whole queue.
        o = out[r:r + 1, 0:ROW_FL]
        o.ap = bass_rust.VecI64Pair([[STRIDE, 16], [1, 1]])
        with nc.allow_non_contiguous_dma(reason="16 x 4B queue-drain marker"):
            return eng.dma_start(out=o, in_=vtile[0:16, 0:1])

    with nc.Block() as block, nc.semaphore("dma_sem") as dma_sem, \
            nc.semaphore("scr_sem") as scr_sem:

        @block.sync
        def _(sync):
            sync.dma_start(out=vtile[:, :], in_=val[:, :]).then_inc(dma_sem, 16)
            sync.wait_ge(dma_sem, 16)
            for r in range(0, half):
                store_row(sync, r).then_inc(scr_sem, 16)
            closer_row(sync, 0).then_inc(dma_sem, 16)
            sync.wait_ge(dma_sem, 48)

        @block.scalar
        def _(scalar):
            scalar.wait_ge(dma_sem, 16)
            for r in range(half, ROWS_PER_CORE):
                store_row(scalar, r).then_inc(scr_sem, 16)
            closer_row(scalar, half).then_inc(dma_sem, 16)
            scalar.wait_ge(dma_sem, 48)

    return nc


def _pack_value(value):
    """[16, C]: partition p<15 = window vflat[8738p : 8738p+8740];
    partition 15 = the 2-float tail (so the closer rewrites real data)."""
    vflat = np.ascontiguousarray(np.asarray(value, np.float32)).reshape(ROW_FL)
    v15 = np.zeros((16, C), np.float32)
    for p in range(NP):
        v15[p] = vflat[p * STRIDE: p * STRIDE + C]
    tail = vflat[NP * STRIDE:]
    v15[15, :tail.size] = tail
    return v15


def kernel(query=None, key=None, value=None, attn_mask=None, **_ignored):
    global LAST_RESULT
    v15 = _pack_value(value)

    nc = build_program()
    core_ids = list(range(N_CORES))
    in_maps = [{"value15": v15} for _ in core_ids]
    res = run_bass_kernel_spmd(nc, in_maps, core_ids, trace=TRACE,
                               **TRACE_KWARGS)
    LAST_RESULT = res

    # Every core's shard is identical (rows don't depend on the row index),
    # but assemble as if sharded: core i supplies rows [i*256, (i+1)*256).
    shards = [res.results[i]["out"].reshape(ROWS_PER_CORE, S, 1, D)
              for i in range(N_CORES)]
    return np.concatenate(shards, axis=0)
